# revision 1
# baseline (speedup 1.0000x reference)
"""Distributed 3-layer GCN on 8 Trainium2 NeuronCores.

Strategy (dst-sharded graph parallelism):
  - Nodes padded N=100000 -> NPAD=100352 = 8 cores * 98 windows * 128.
  - Each core owns a contiguous 12544-node dst shard; edges are routed to the
    core owning their destination, grouped by (dst window of 128, src chunk of
    25088), padded so every core runs the identical instruction stream.
  - Per layer: node features (with n_out folded in, bf16) are AllGathered to a
    full [NPAD, 512] table per core; edge messages are fetched with dma_gather
    (one 1KB row per edge) and segment-summed into PSUM via one-hot matmuls
    S_w^T[edge, dstslot] @ X[edge, :]; self-loop rows added from the local
    shard; the dense @W runs as 4 K=128 matmuls on the transposed aggregate,
    with the bias applied as a rank-1 (1/n_in (x) b) matmul so a single
    per-partition scale n_in*n_out plus LeakyReLU finishes the layer.
  - Final layer folds pred_W into W2 (512x1) and emits logits directly.
"""
import numpy as np
import ml_dtypes

import concourse.bacc as bacc_mod
import concourse.bass as bass
import concourse.mybir as mybir
import concourse.tile as tile
from concourse.bass_utils import run_bass_kernel_spmd
from concourse.masks import make_identity

BF16 = np.float16  # fp16: same 2B as bf16, 8x mantissa (logits are cancellation-heavy)

N = 100000
E = 3200000
D = 512
NEG = 0.01
NC = 8
P = 128
REAL_PC = 12500          # real nodes per core
PC = 12544               # padded nodes per core (98 * 128)
W = PC // P              # 98 windows per core
NCHUNK = 4
CS = 25088               # chunk size (< 32768 for int16 gather idxs)
NPAD = NC * PC


def _host_prep(weight, significance, src, dst, emb_table, lin_W, lin_b,
               W0, b0, W1, b1, W2, b2, pred_W, pred_b):
    """Build all per-core device inputs. Returns (in_maps, meta)."""
    f32 = np.float32
    src = np.asarray(src); dst = np.asarray(dst)

    # ---- padded node ids: real node n -> core n//12500, local n%12500
    def pad_id(n):
        return (n // REAL_PC) * PC + (n % REAL_PC)

    src_p = pad_id(src.astype(np.int64))
    dst_p = pad_id(dst.astype(np.int64))

    core_e = dst_p // PC
    w_e = (dst_p % PC) // P
    slot_e = dst_p % P
    chunk_e = src_p // CS
    idx_e = (src_p % CS).astype(np.int16)

    # ---- group edges by (core, window, chunk)
    key = ((core_e * W + w_e) * NCHUNK + chunk_e).astype(np.int64)
    counts = np.bincount(key, minlength=NC * W * NCHUNK).reshape(NC, W, NCHUNK)
    cnt_wc = counts.max(axis=0)                       # [W, NCHUNK]
    cnt_wc = ((cnt_wc + P - 1) // P) * P              # pad to tile multiple
    t_wc = cnt_wc // P                                # tiles per (w, chunk)
    T_w = t_wc.sum(axis=1)                            # tiles per window
    T_total = int(T_w.sum())
    IDXF = int(cnt_wc.sum()) // 16                    # idx free dim

    # capacity-based offsets, shared by all cores
    cap_flat = cnt_wc.reshape(-1)                     # [W*NCHUNK]
    grp_off = np.concatenate([[0], np.cumsum(cap_flat)])[:-1]  # edge-slot offset

    order = np.argsort(key, kind="stable")
    key_s = key[order]
    # position of each edge within its group
    grp_start_per_edge = np.searchsorted(key_s, key_s)  # first occurrence index
    pos_in_grp = np.arange(E) - grp_start_per_edge

    # ---- degrees / norms on padded ids
    deg_out = np.bincount(src, minlength=N).astype(f32) + 1.0
    deg_in = np.bincount(dst, minlength=N).astype(f32) + 1.0
    n_out = 1.0 / np.sqrt(deg_out)
    n_in = 1.0 / np.sqrt(deg_in)
    n_out_p = np.ones(NPAD, f32); n_in_p = np.ones(NPAD, f32)
    ids = pad_id(np.arange(N, dtype=np.int64))
    n_out_p[ids] = n_out; n_in_p[ids] = n_in
    inv_n_in_p = np.ones(NPAD, f32)
    inv_n_in_p[ids] = np.sqrt(deg_in)

    # ---- node input features [NPAD, 3]
    x0 = np.zeros((NPAD, 3), f32)
    x0[ids, 0] = np.asarray(weight, f32)
    x0[ids, 1:] = np.asarray(emb_table, f32)[np.asarray(significance)]

    # ---- weights (bf16 device layouts)
    def w_chunks(Wm):  # [512, 512] -> [128, 4*512] with chunk k at cols k*512:
        out = np.empty((P, 4 * D), BF16)
        for k in range(4):
            out[:, k * D:(k + 1) * D] = Wm[k * P:(k + 1) * P, :].astype(BF16)
        return out

    Wp = (np.asarray(W2, np.float64) @ np.asarray(pred_W, np.float64)).astype(f32)  # [512,1]
    bp = float((np.asarray(b2, np.float64) @ np.asarray(pred_W, np.float64)
                + np.asarray(pred_b, np.float64)).reshape(()))

    shared = {
        "w0c": w_chunks(np.asarray(W0)),
        "w1c": w_chunks(np.asarray(W1)),
        "wpc": Wp.reshape(4, P).T.astype(BF16).copy(),      # [128, 4], chunk k at col k
        "linw": np.asarray(lin_W, f32).astype(BF16),        # [3, 512]
        "linb": np.asarray(lin_b, f32).astype(BF16).reshape(1, D),
        "b0r": np.asarray(b0, f32).astype(BF16).reshape(1, D),
        "b1r": np.asarray(b1, f32).astype(BF16).reshape(1, D),
        "bpr": np.full((1, 1), bp, BF16),
    }

    # ---- per-core tensors
    in_maps = []
    for c in range(NC):
        mask = core_e == c
        eo = order[mask[order]]                      # this core's edges, grouped
        kk = key_s[mask[order]] - c * W * NCHUNK     # local group id [0, W*NCHUNK)
        pos = pos_in_grp[mask[order]]
        eslot = grp_off[kk] + pos                    # padded edge slot

        # idx array [128, IDXF] int16 (16-partition wrap, replicated x8), pad=0
        idx_flat = np.zeros(int(cnt_wc.sum()), np.int16)
        idx_flat[eslot] = idx_e[eo]
        idx_l = np.zeros((16, IDXF), np.int16)
        i_all = np.arange(int(cnt_wc.sum()))
        idx_l[i_all % 16, i_all // 16] = idx_flat
        idx_l = np.tile(idx_l, (8, 1))

        # dstslot [128, T_total] bf16 (255 for pad); per (w,c) group local tiles
        slot_flat = np.full(int(cnt_wc.sum()), 255, np.int32)
        slot_flat[eslot] = slot_e[eo]
        # map edge slot -> (partition, tile col)
        grp_of_slot = np.repeat(np.arange(W * NCHUNK), cap_flat)
        j_in_grp = i_all - grp_off[grp_of_slot]
        tile_base = np.concatenate([[0], np.cumsum(t_wc.reshape(-1))])[:-1]
        tcol = tile_base[grp_of_slot] + j_in_grp // P
        dslot = np.zeros((P, T_total), BF16)
        dslot[j_in_grp % P, tcol] = slot_flat.astype(BF16)

        # norm scale columns [128, W]
        def col(a):
            return a[c * PC:(c + 1) * PC].reshape(W, P).T.copy()
        s0 = col(n_out_p)                            # layer-0 readout scale
        s12 = col(n_in_p * n_out_p)                  # mid-layer readout scale
        s3 = col(n_in_p)                             # final readout scale
        invn = inv_n_in_p[c * PC:(c + 1) * PC].astype(BF16).reshape(1, PC)

        x0t = x0[c * PC:(c + 1) * PC].T.astype(BF16).copy()  # [3, 12544]

        m = {"idxs": idx_l, "dslot": dslot, "s0": s0, "s12": s12, "s3": s3,
             "invn": invn, "x0t": x0t}
        m.update(shared)
        in_maps.append(m)

    meta = dict(cnt_wc=cnt_wc, t_wc=t_wc, T_w=T_w, T_total=T_total, IDXF=IDXF)
    return in_maps, meta


def _build(meta):
    import os
    cnt_wc = meta["cnt_wc"]; t_wc = meta["t_wc"]; T_w = meta["T_w"]
    T_total = meta["T_total"]; IDXF = meta["IDXF"]
    T_max = int(T_w.max())
    DBG_W = int(os.environ.get("KW", W))     # windows per layer (debug)
    DBG_L = int(os.environ.get("KL", 3))     # layers (debug)

    nc = bacc_mod.Bacc(num_devices=NC)
    dt = mybir.dt

    # ---- IO
    idxs_in = nc.dram_tensor("idxs", [P, IDXF], dt.int16, kind="ExternalInput")
    dslot_in = nc.dram_tensor("dslot", [P, T_total], dt.float16, kind="ExternalInput")
    s0_in = nc.dram_tensor("s0", [P, W], dt.float32, kind="ExternalInput")
    s12_in = nc.dram_tensor("s12", [P, W], dt.float32, kind="ExternalInput")
    s3_in = nc.dram_tensor("s3", [P, W], dt.float32, kind="ExternalInput")
    invn_in = nc.dram_tensor("invn", [1, PC], dt.float16, kind="ExternalInput")
    x0t_in = nc.dram_tensor("x0t", [3, PC], dt.float16, kind="ExternalInput")
    w0c_in = nc.dram_tensor("w0c", [P, 4 * D], dt.float16, kind="ExternalInput")
    w1c_in = nc.dram_tensor("w1c", [P, 4 * D], dt.float16, kind="ExternalInput")
    wpc_in = nc.dram_tensor("wpc", [P, 4], dt.float16, kind="ExternalInput")
    linw_in = nc.dram_tensor("linw", [3, D], dt.float16, kind="ExternalInput")
    linb_in = nc.dram_tensor("linb", [1, D], dt.float16, kind="ExternalInput")
    b0r_in = nc.dram_tensor("b0r", [1, D], dt.float16, kind="ExternalInput")
    b1r_in = nc.dram_tensor("b1r", [1, D], dt.float16, kind="ExternalInput")
    bpr_in = nc.dram_tensor("bpr", [1, 1], dt.float16, kind="ExternalInput")
    logits_out = nc.dram_tensor("logits", [PC, 1], dt.float32, kind="ExternalOutput")

    # ---- internal DRAM
    h_loc = [nc.dram_tensor(f"h_loc{l}", [PC, D], dt.float16, kind="Internal")
             for l in range(3)]
    h_full = [nc.dram_tensor(f"h_full{l}", [NPAD, D], dt.float16,
                             kind="Internal", addr_space="Shared")
              for l in range(3)]

    with tile.TileContext(nc) as tc:
        with tc.tile_pool(name="const", bufs=1) as cp, \
             tc.tile_pool(name="xp", bufs=2) as xp, \
             tc.tile_pool(name="sp", bufs=2) as sp, \
             tc.tile_pool(name="small", bufs=3) as smp, \
             tc.tile_pool(name="psA", bufs=2, space="PSUM") as psA, \
             tc.tile_pool(name="psB", bufs=2, space="PSUM") as psB, \
             tc.tile_pool(name="psT", bufs=2, space="PSUM") as psT:

            # ---- constants
            ident = cp.tile([P, P], dt.float16)
            make_identity(nc, ident[:])
            iota16 = cp.tile([P, T_max * P], dt.int16)
            nc.gpsimd.iota(iota16[:], pattern=[[0, T_max], [1, P]], base=0,
                           channel_multiplier=0)
            iota_b = cp.tile([P, T_max * P], dt.float16)
            nc.vector.tensor_copy(iota_b[:], iota16[:])
            ones_r = cp.tile([1, P], dt.float16)
            nc.vector.memset(ones_r[:], 1.0)

            s0_t = cp.tile([P, W], dt.float32)
            nc.sync.dma_start(out=s0_t[:], in_=s0_in[:, :])
            s12_t = cp.tile([P, W], dt.float32)
            nc.sync.dma_start(out=s12_t[:], in_=s12_in[:, :])
            s3_t = cp.tile([P, W], dt.float32)
            nc.sync.dma_start(out=s3_t[:], in_=s3_in[:, :])
            invn_t = cp.tile([1, PC], dt.float16)
            nc.sync.dma_start(out=invn_t[:], in_=invn_in[:, :])
            x0t_t = cp.tile([3, PC], dt.float16)
            nc.sync.dma_start(out=x0t_t[:], in_=x0t_in[:, :])
            w0_t = cp.tile([P, 4 * D], dt.float16)
            nc.sync.dma_start(out=w0_t[:], in_=w0c_in[:, :])
            w1_t = cp.tile([P, 4 * D], dt.float16)
            nc.sync.dma_start(out=w1_t[:], in_=w1c_in[:, :])
            wp_t = cp.tile([P, 4], dt.float16)
            nc.sync.dma_start(out=wp_t[:], in_=wpc_in[:, :])
            linw_t = cp.tile([3, D], dt.float16)
            nc.sync.dma_start(out=linw_t[:], in_=linw_in[:, :])
            linb_t = cp.tile([1, D], dt.float16)
            nc.sync.dma_start(out=linb_t[:], in_=linb_in[:, :])
            b0_t = cp.tile([1, D], dt.float16)
            nc.sync.dma_start(out=b0_t[:], in_=b0r_in[:, :])
            b1_t = cp.tile([1, D], dt.float16)
            nc.sync.dma_start(out=b1_t[:], in_=b1r_in[:, :])
            bp_t = cp.tile([1, 1], dt.float16)
            nc.sync.dma_start(out=bp_t[:], in_=bpr_in[:, :])

            logits_sb = cp.tile([P, W], dt.float32)

            # ---- layer 0 input: x~0 = (X0 @ linW + linb) * n_out  -> h_loc[0]
            for w in range(int(os.environ.get("KF", W))):
                ps = psB.tile([P, D], dt.float32, space="PSUM")
                nc.tensor.matmul(out=ps[:], lhsT=x0t_t[:, w * P:(w + 1) * P],
                                 rhs=linw_t[:], start=True, stop=False)
                nc.tensor.matmul(out=ps[:], lhsT=ones_r[:], rhs=linb_t[:],
                                 start=False, stop=True)
                ht = smp.tile([P, D], dt.float16, tag="h0w")
                nc.scalar.activation(ht[:], ps[:],
                                     mybir.ActivationFunctionType.Copy,
                                     bias=0.0, scale=s0_t[:, w:w + 1])
                nc.sync.dma_start(out=h_loc[0][w * P:(w + 1) * P, :], in_=ht[:])

            # ---- per-layer pipeline
            idx_off_w = np.concatenate([[0], np.cumsum(cnt_wc.sum(axis=1) // 16)])
            tile_off_w = np.concatenate([[0], np.cumsum(T_w)])

            for l in range(DBG_L):
                last = l == 2
                w_t = [w0_t, w1_t, wp_t][l]
                b_t = [b0_t, b1_t, bp_t][l]
                s_t = [s12_t, s12_t, s3_t][l]
                nout = 1 if last else D

                nc.gpsimd.collective_compute(
                    "AllGather", mybir.AluOpType.bypass,
                    replica_groups=[list(range(NC))],
                    ins=[h_loc[l][:, :]], outs=[h_full[l][:, :]],
                )

                for w in range(DBG_W):
                    Tw = int(T_w[w])
                    # -- load idx + dstslot for this window
                    idx_t = smp.tile([P, int(idx_off_w[w + 1] - idx_off_w[w])],
                                     dt.int16, tag="idx")
                    nc.sync.dma_start(
                        out=idx_t[:],
                        in_=idxs_in[:, int(idx_off_w[w]):int(idx_off_w[w + 1])])
                    ds_t = smp.tile([P, Tw], dt.float16, tag="ds")
                    nc.sync.dma_start(
                        out=ds_t[:],
                        in_=dslot_in[:, int(tile_off_w[w]):int(tile_off_w[w + 1])])

                    # -- gather edge source rows (4 chunks)
                    x_t = xp.tile([P, Tw * D], dt.float16, tag="x")
                    coff = 0   # idx offset within window (int16 elements)
                    toff = 0   # tile offset within window
                    for ch in range(NCHUNK):
                        rem = int(cnt_wc[w][ch])
                        while rem > 0:           # dma_gather caps at 1024 idxs
                            cnt = min(rem, 1024)
                            nc.gpsimd.dma_gather(
                                x_t[:, toff * D:(toff + cnt // P) * D]
                                    .rearrange("p (t d) -> p t d", d=D),
                                h_full[l][ch * CS:(ch + 1) * CS, :],
                                idx_t[:, coff // 16:(coff + cnt) // 16],
                                cnt, cnt, D, elem_step=D,
                            )
                            coff += cnt
                            toff += cnt // P
                            rem -= cnt

                    # -- one-hot S^T tiles
                    s_oh = sp.tile([P, Tw * P], dt.float16, tag="soh")
                    nc.vector.tensor_tensor(
                        out=s_oh[:],
                        in0=ds_t[:, :, None].to_broadcast([P, Tw, P]),
                        in1=iota_b[:, :Tw * P],
                        op=mybir.AluOpType.is_equal,
                    )

                    # -- segment-sum matmuls
                    agg_ps = psA.tile([P, D], dt.float32, space="PSUM")
                    for t in range(Tw):
                        nc.tensor.matmul(
                            out=agg_ps[:],
                            lhsT=s_oh[:, t * P:(t + 1) * P],
                            rhs=x_t[:, t * D:(t + 1) * D],
                            start=(t == 0), stop=(t == Tw - 1),
                        )

                    # -- add self-loop rows (local shard, already n_out-scaled)
                    self_t = smp.tile([P, D], dt.float16, tag="self")
                    nc.sync.dma_start(out=self_t[:],
                                      in_=h_loc[l][w * P:(w + 1) * P, :])
                    agg_sb = smp.tile([P, D], dt.float16, tag="agg")
                    nc.vector.tensor_tensor(out=agg_sb[:], in0=agg_ps[:],
                                            in1=self_t[:],
                                            op=mybir.AluOpType.add)

                    # -- transpose agg (4 x [128,128])
                    tr_ps = psT.tile([P, D], dt.float16, space="PSUM")
                    for k in range(4):
                        nc.tensor.transpose(
                            out=tr_ps[:, k * P:(k + 1) * P],
                            in_=agg_sb[:, k * P:(k + 1) * P],
                            identity=ident[:])
                    aggT = smp.tile([P, D], dt.float16, tag="aggT")
                    nc.vector.tensor_copy(aggT[:], tr_ps[:])

                    # -- dense: psum = aggT.T @ W  (+ rank-1 bias (1/n_in) (x) b)
                    dps = psB.tile([P, nout], dt.float32, space="PSUM")
                    for k in range(4):
                        nc.tensor.matmul(
                            out=dps[:],
                            lhsT=aggT[:, k * P:(k + 1) * P],
                            rhs=w_t[:, k * nout:(k + 1) * nout],
                            start=(k == 0), stop=False)
                    nc.tensor.matmul(out=dps[:],
                                     lhsT=invn_t[:, w * P:(w + 1) * P],
                                     rhs=b_t[:], start=False, stop=True)

                    # -- readout
                    if last:
                        nc.scalar.activation(logits_sb[:, w:w + 1], dps[:],
                                             mybir.ActivationFunctionType.Copy,
                                             bias=0.0, scale=s_t[:, w:w + 1])
                    else:
                        th = smp.tile([P, D], dt.float16, tag="th")
                        nc.scalar.activation(th[:], dps[:],
                                             mybir.ActivationFunctionType.Copy,
                                             bias=0.0, scale=s_t[:, w:w + 1])
                        tu = smp.tile([P, D], dt.float16, tag="tu")
                        nc.vector.tensor_scalar_mul(tu[:], th[:], NEG)
                        hn = smp.tile([P, D], dt.float16, tag="hn")
                        nc.vector.tensor_tensor(out=hn[:], in0=th[:], in1=tu[:],
                                                op=mybir.AluOpType.max)
                        nc.sync.dma_start(
                            out=h_loc[l + 1][w * P:(w + 1) * P, :], in_=hn[:])

            # ---- write logits
            if DBG_L == 3 and DBG_W == W:
                nc.sync.dma_start(
                    out=logits_out[:, 0:1].rearrange("(w p) o -> p (w o)", p=P),
                    in_=logits_sb[:])
            else:  # debug: keep logits tensor written
                zt = cp.tile([P, 1], dt.float32)
                nc.vector.memset(zt[:], 0.0)
                nc.sync.dma_start(out=logits_out[0:P, :], in_=zt[:])

    nc.finalize()
    return nc


def kernel(**inputs) -> np.ndarray:
    in_maps, meta = _host_prep(**inputs)
    nc = _build(meta)
    res = run_bass_kernel_spmd(nc, in_maps, core_ids=list(range(NC)))
    out = np.empty((N, 1), np.float32)
    for c in range(NC):
        out[c * REAL_PC:(c + 1) * REAL_PC] = res.results[c]["logits"][:REAL_PC]
    return out

